# revision 1
# baseline (speedup 1.0000x reference)
"""Trainium2 Bass kernel for nn_Attention_org_cross_85074712199395.

Reference computes two fully independent cross-attention branches:
  branch 0: Q(emb1;Wq)   x Kd,Vd(emb_alld;Wkd0,Wvd0) -> O1  via Wout
  branch 1: Qd(embd1;Wqd) x K,V  (emb_all; Wk0, Wv0 ) -> Od1 via Woutd
Sharding: 8 cores = 4 batches x 2 branches. Zero collectives.

Per-core math (one batch b, one branch), N=4096 tokens, H=4 heads, 256 ch/head:
  Q[h]  = emb_q[:, h]  @ WqT[h]      [N,256]   (n on partitions)
  Kd[h] = emb_kv[:, h] @ WkT         [N,256]
  s[h]  = sum_n Q[n,c] Kd[n,k]       [256,256]  (PSUM-accumulated over N)
  p[h]  = softmax(inorm(s[h]*scale)) -- scale folded into eps' = eps*KV
  VdT[h]= Wv @ emb_kv[:, h].T        [256,N]   (produced pre-transposed)
  ctxT[(h,c), n] = sum_k p[h][c,k] VdT[h][k,n]
  O[n,o] = sum_C ctxT[C,n] WoutT_perm[C,o]     (host permutes Wout.T to
                                                head-major C = h*256+c order)
All matmul operands are fp16 (10-bit mantissa, full PE rate + FWL; fp32
accumulation in PSUM). Inputs are cast to fp16 on the host, halving DMA.
"""

import sys
import types

import numpy as np

B, N, C, KV, H = 4, 4096, 1024, 1024, 4
Ch = C // H          # 256
EPS_ADJ = 1e-5 * KV  # InstanceNorm eps with the 1/sqrt(KV) score scale folded in
NB = N // 512        # token blocks


def _ensure_axon_hooks():
    """Inject antenv.axon_hooks (absent in this image) so trace=True works."""
    if "antenv.axon_hooks" in sys.modules:
        return
    try:
        import antenv  # noqa: F401
    except ImportError:
        return
    mod = types.ModuleType("antenv.axon_hooks")
    state = [None]
    mod.set_axon_ntff_profile_hook = lambda h: state.__setitem__(0, h)
    mod.get_axon_ntff_profile_hook = lambda: state[0]
    sys.modules["antenv.axon_hooks"] = mod
    try:
        from trn_agent_boot.trn_boot import _ntff_profile_via_ctypes

        mod.set_axon_ntff_profile_hook(
            _ntff_profile_via_ctypes("/opt/axon/libaxon_pjrt.so")
        )
    except Exception:
        pass


def build_nc(n_tokens=N):
    """Build + compile the per-core Bass program (SPMD-identical on all cores)."""
    import concourse.bass as bass
    import concourse.mybir as mybir
    import concourse.tile as tile
    from concourse import bacc
    from concourse.masks import make_identity

    f32 = mybir.dt.float32
    f16 = mybir.dt.float16
    f16d = mybir.dt.float16
    Exp = mybir.ActivationFunctionType.Exp
    X = mybir.AxisListType.X
    mult = mybir.AluOpType.mult
    add = mybir.AluOpType.add
    nb = n_tokens // 512

    nc = bacc.Bacc("TRN2", target_bir_lowering=False, debug=False, num_devices=8)

    embq_d = nc.dram_tensor("embq", [128, 8, n_tokens], f16d, kind="ExternalInput").ap()
    embkv_d = nc.dram_tensor("embkv", [128, 8, n_tokens], f16d, kind="ExternalInput").ap()
    wq_d = nc.dram_tensor("wq", [128, 8, 256], f16d, kind="ExternalInput").ap()
    wk_d = nc.dram_tensor("wk", [128, 2, 256], f16d, kind="ExternalInput").ap()
    wv_d = nc.dram_tensor("wv", [128, 4, 128], f16d, kind="ExternalInput").ap()
    wout_d = nc.dram_tensor("wout", [128, 8, 1024], f16d, kind="ExternalInput").ap()
    out_d = nc.dram_tensor("out", [n_tokens, 1024], f32, kind="ExternalOutput").ap()

    with tile.TileContext(nc) as tc:
        with tc.tile_pool(name="weights", bufs=1) as wpool, \
             tc.tile_pool(name="pt", bufs=1) as pt_pool, \
             tc.tile_pool(name="den", bufs=1) as den_pool, \
             tc.tile_pool(name="stat", bufs=1) as stat_pool:

            # ---- resident weights: everything fp16, plain HWDGE loads ----
            wq = wpool.tile([128, 8, 256], f16)
            nc.sync.dma_start(wq[:], wq_d[:])
            wk = wpool.tile([128, 2, 256], f16)
            nc.sync.dma_start(wk[:], wk_d[:])
            wv = wpool.tile([128, 4, 128], f16)
            nc.sync.dma_start(wv[:], wv_d[:])
            wout = wpool.tile([128, 8, 1024], f16)
            ident = wpool.tile([128, 128], f16)
            make_identity(nc, ident[:])
            ones_col = wpool.tile([128, 1], f32)
            nc.vector.memset(ones_col[:], 1.0)
            ones_row = wpool.tile([1, 128], f32)
            nc.vector.memset(ones_row[:], 1.0)
            warm_sb = wpool.tile([128, 128], f32)
            nc.vector.memset(warm_sb[:], 0.0)

            pts = [pt_pool.tile([128, 256], f16, name=f"pt{i}", tag=f"pt{i}") for i in range(8)]  # (h,kc)
            dens = [den_pool.tile([128, 4], f32, name=f"den{i}", tag=f"den{i}") for i in range(4)]   # per h

            def vd_project(ekv_tile, ps_pool, sb_pool):
                """VdT for one 512-token group: 16 MMs + 8 fp16 evacuations."""
                vds = []
                for h in range(H):
                    for kc in range(2):
                        vp = ps_pool.tile([128, 512], f32, name="vp", tag="vp")
                        for i, ccl in enumerate((0, 1)):
                            nc.tensor.matmul(vp[:], wv[:, ccl * 2 + kc, :],
                                             ekv_tile[:, 2 * h + ccl, :],
                                             start=(i == 0), stop=(i == 1))
                        vs = sb_pool.tile([128, 512], f16, name="vs", tag="vs")
                        nc.vector.tensor_copy(vs[:], vp[:])
                        vds.append(vs)
                return vds

            # ================= phase A: Q/Kd projections + scores =========
            with tc.tile_pool(name="ekvB", bufs=3) as ekvB_pool, \
                 tc.tile_pool(name="vd_sb", bufs=20) as vd_sb, \
                 tc.tile_pool(name="s_scope", bufs=1) as _unused_scope:
              with tc.tile_pool(name="s_ps", bufs=1, space="PSUM") as s_pool:
                s_tiles = [s_pool.tile([128, 512], f32, name=f"s{i}", tag=f"s{i}") for i in range(4)]

                with tc.tile_pool(name="embA", bufs=6) as embq_pool, \
                     tc.tile_pool(name="ekvA", bufs=6) as embkv_pool, \
                     tc.tile_pool(name="qkd_ps", bufs=3, space="PSUM") as qkd_ps, \
                     tc.tile_pool(name="warm_ps", bufs=1, space="PSUM") as warm_pool, \
                     tc.tile_pool(name="qkd_sb", bufs=6) as qkd_sb:
                    # keep PE busy (HAM warm) while the first emb blocks stream in
                    wps = warm_pool.tile([128, 512], f32)
                    for w in range(12):
                        nc.tensor.matmul(wps[:, 0:128], warm_sb[:], warm_sb[:],
                                         start=(w == 0), stop=(w == 11))
                    for blk in range(nb):
                        sl = slice(blk * 512, (blk + 1) * 512)
                        eq = embq_pool.tile([128, 8, 512], f16)
                        ekv = embkv_pool.tile([128, 8, 512], f16)
                        if blk == 0:
                            # halve the first transfers so head-0/1 matmuls can
                            # start before the whole block lands (subtile deps)
                            nc.sync.dma_start(eq[:, 0:4, :], embq_d[:, 0:4, sl])
                            nc.sync.dma_start(ekv[:, 0:4, :], embkv_d[:, 0:4, sl])
                            nc.sync.dma_start(eq[:, 4:8, :], embq_d[:, 4:8, sl])
                            nc.sync.dma_start(ekv[:, 4:8, :], embkv_d[:, 4:8, sl])
                        else:
                            nc.sync.dma_start(eq[:], embq_d[:, :, sl])
                            nc.sync.dma_start(ekv[:], embkv_d[:, :, sl])
                        for h in range(H):
                            for ns in range(4):
                                qk = qkd_ps.tile([128, 512], f32)
                                for i, ccl in enumerate((0, 1)):
                                    lhs = eq[:, 2 * h + ccl, ns * 128:(ns + 1) * 128]
                                    nc.tensor.matmul(qk[:, 0:256], lhs, wq[:, h * 2 + ccl, :],
                                                     start=(i == 0), stop=(i == 1))
                                for i, ccl in enumerate((0, 1)):
                                    lhs = ekv[:, 2 * h + ccl, ns * 128:(ns + 1) * 128]
                                    nc.tensor.matmul(qk[:, 256:512], lhs, wk[:, ccl, :],
                                                     start=(i == 0), stop=(i == 1))
                                qks = qkd_sb.tile([128, 512], f16)
                                nc.vector.tensor_copy(qks[:], qk[:])
                                first = blk == 0 and ns == 0
                                last = blk == nb - 1 and ns == 3
                                for ch in range(2):
                                    # one accumulation group per PSUM bank: start
                                    # clears has_written for the WHOLE bank, so only
                                    # the first matmul into the tile may set it
                                    nc.tensor.matmul(
                                        s_tiles[h][:, ch * 256:(ch + 1) * 256],
                                        qks[:, ch * 128:(ch + 1) * 128],
                                        qks[:, 256:512],
                                        start=first and ch == 0,
                                        stop=last and ch == 1)

                # wout is first needed ~25us into phase B; keep it out of the
                # startup DMA window
                nc.sync.dma_start(wout[:], wout_d[:])
                # prefetch the first two phase-B token groups; the SWDGE queue
                # is otherwise idle from here
                ekv_b0 = ekvB_pool.tile([128, 8, 512], f16, name="ekv_b", tag="ekv_b")
                nc.sync.dma_start(ekv_b0[:], embkv_d[:, :, 0:512])
                ekv_b1 = None
                if nb > 1:
                    ekv_b1 = ekvB_pool.tile([128, 8, 512], f16, name="ekv_b", tag="ekv_b")
                    nc.sync.dma_start(ekv_b1[:], embkv_d[:, :, 512:1024])

                # ============= softmax + inorm + P^T ======================
                vds0 = None
                vds1 = None
                with tc.tile_pool(name="tot_ps", bufs=1, space="PSUM") as tot_ps_pool, \
                     tc.tile_pool(name="pt_ps", bufs=1, space="PSUM") as pt_ps_pool, \
                     tc.tile_pool(name="vd0_ps", bufs=2, space="PSUM") as vd0_ps_pool, \
                     tc.tile_pool(name="p_sb", bufs=4) as p_pool, \
                     tc.tile_pool(name="stat4", bufs=4) as stat4_pool, \
                     tc.tile_pool(name="scr", bufs=1) as scr_pool:
                    scratch = scr_pool.tile([128, 512], f32)
                    # --- stacked stats for all 4 heads ---
                    stat_all = stat4_pool.tile([128, 16], f32)
                    for h in range(H):
                        for cc in range(2):
                            s_ap = s_tiles[h][:, cc * 256:(cc + 1) * 256]
                            nc.vector.reduce_sum(stat_all[:, h * 4 + cc:h * 4 + cc + 1],
                                                 s_ap, axis=X)
                            nc.scalar.activation(
                                scratch[:, cc * 256:(cc + 1) * 256], s_ap,
                                mybir.ActivationFunctionType.Square,
                                accum_out=stat_all[:, h * 4 + 2 + cc:h * 4 + 3 + cc])
                    tb = tot_ps_pool.tile([128, 16], f32, name="tb", tag="tb")
                    nc.tensor.matmul(tb[:1, 0:16], ones_col[:], stat_all[:],
                                     start=True, stop=True)
                    sc = stat4_pool.tile([1, 40], f32)
                    nc.vector.tensor_copy(sc[:1, 0:16], tb[:1, 0:16])
                    inv = 1.0 / (256.0 * 256.0)
                    # cols h*4+{0,1,2,3} = sum0,sum1,sq0,sq1
                    # -> [msum,qsum]x4 at 16:24, [mean,Ex2]x4 at 24:32
                    nc.vector.tensor_add(sc[:1, 16:24], sc[:1, 0:16:2], sc[:1, 1:16:2])
                    nc.vector.tensor_scalar_mul(sc[:1, 24:32], sc[:1, 16:24], inv)
                    nc.vector.tensor_mul(sc[:1, 32:36], sc[:1, 24:32:2], sc[:1, 24:32:2])
                    nc.vector.tensor_sub(sc[:1, 36:40], sc[:1, 25:32:2], sc[:1, 32:36])
                    sc2 = stat4_pool.tile([1, 16], f32)
                    nc.vector.tensor_scalar_add(sc2[:1, 0:4], sc[:1, 36:40], EPS_ADJ)
                    nc.scalar.sqrt(sc2[:1, 4:8], sc2[:1, 0:4])
                    nc.vector.reciprocal(sc2[:1, 8:12], sc2[:1, 4:8])          # rs x4
                    nc.vector.tensor_mul(sc2[:1, 12:16], sc[:1, 24:32:2], sc2[:1, 8:12])
                    nc.vector.tensor_scalar_mul(sc2[:1, 12:16], sc2[:1, 12:16], -1.0)
                    # broadcast [rs x4, -rs*m x4] to all partitions via PE
                    nc.tensor.matmul(tb[:, 0:8], ones_row[:], sc2[:1, 8:16],
                                     start=True, stop=True)
                    bc = stat4_pool.tile([128, 8], f32)
                    nc.vector.tensor_copy(bc[:], tb[:, 0:8])
                    for h in range(H):
                        p = p_pool.tile([128, 512], f16)
                        den = dens[h]
                        for cc in range(2):
                            nc.scalar.activation(
                                p[:, cc * 256:(cc + 1) * 256],
                                s_tiles[h][:, cc * 256:(cc + 1) * 256],
                                Exp, bias=bc[:, 4 + h:5 + h], scale=bc[:, h:h + 1],
                                accum_out=den[:, cc:cc + 1])
                            nc.vector.reciprocal(den[:, 2 + cc:3 + cc], den[:, cc:cc + 1])
                        for kc in range(2):
                            ptp = pt_ps_pool.tile([128, 256], f16)
                            for cc in range(2):
                                nc.tensor.transpose(
                                    ptp[:, cc * 128:(cc + 1) * 128],
                                    p[:, cc * 256 + kc * 128: cc * 256 + (kc + 1) * 128],
                                    ident[:])
                            nc.vector.tensor_copy(pts[h * 2 + kc][:], ptp[:])
                        if h == 0:
                            # overlap: g=0/1 VdT projections are softmax-independent
                            vds0 = vd_project(ekv_b0, vd0_ps_pool, vd_sb)
                        if h == 2 and nb > 1:
                            vds1 = vd_project(ekv_b1, vd0_ps_pool, vd_sb)

              # ================= phase B: ctx, out ======================
              if True:
                with tc.tile_pool(name="vd_ps", bufs=2, space="PSUM") as vd_ps, \
                     tc.tile_pool(name="ctx_ps", bufs=3, space="PSUM") as ctx_ps, \
                     tc.tile_pool(name="ctx_sb", bufs=16) as ctx_sb, \
                     tc.tile_pool(name="o_ps", bufs=3, space="PSUM") as o_ps, \
                     tc.tile_pool(name="o_sb", bufs=6) as o_sb:
                    for g in range(nb):
                        if g == 0:
                            vds = vds0
                        elif g == 1 and vds1 is not None:
                            vds = vds1
                        else:
                            ekv = ekvB_pool.tile([128, 8, 512], f16, name="ekv_b", tag="ekv_b")
                            nc.sync.dma_start(ekv[:], embkv_d[:, :, g * 512:(g + 1) * 512])
                            vds = vd_project(ekv, vd_ps, vd_sb)
                        ctxs = []
                        for h in range(H):
                            for ccl in range(2):
                                cp = ctx_ps.tile([128, 512], f32)
                                for i, kc in enumerate((0, 1)):
                                    nc.tensor.matmul(
                                        cp[:],
                                        pts[h * 2 + kc][:, ccl * 128:(ccl + 1) * 128],
                                        vds[h * 2 + kc][:],
                                        start=(i == 0), stop=(i == 1))
                                cs = ctx_sb.tile([128, 512], f16)
                                nc.vector.tensor_scalar_mul(cs[:], cp[:],
                                                            dens[h][:, 2 + ccl:3 + ccl])
                                ctxs.append(cs)
                        for ns in range(4):
                            ot = o_sb.tile([128, 1024], f32)
                            for oh in range(2):
                                op = o_ps.tile([128, 512], f32)
                                for j in range(8):
                                    nc.tensor.matmul(
                                        op[:], ctxs[j][:, ns * 128:(ns + 1) * 128],
                                        wout[:, j, oh * 512:(oh + 1) * 512],
                                        start=(j == 0), stop=(j == 7))
                                nc.vector.tensor_copy(ot[:, oh * 512:(oh + 1) * 512], op[:])
                            r0 = g * 512 + ns * 128
                            nc.sync.dma_start(out_d[r0:r0 + 128, :], ot[:])

    nc.compile()
    return nc


# ---------------- host-side data prep ----------------

def _prep_embT(e):
    # [nt, 1024] -> [128, 8, nt]: partition p, chunk cc -> channel cc*128+p
    return np.ascontiguousarray(
        e.T.reshape(8, 128, -1).transpose(1, 0, 2).astype(np.float16))


def _prep_wq(Wq):
    # [H, o, c] -> WqT [h, c, o] -> [128, (h,cc), 256]
    WqT = Wq.transpose(0, 2, 1)
    return np.ascontiguousarray(
        WqT.reshape(4, 2, 128, 256).transpose(2, 0, 1, 3).reshape(128, 8, 256)
        .astype(np.float16))


def _prep_wk(Wk):
    # [k, c] -> T [c, k] -> [128, cc, 256]
    return np.ascontiguousarray(
        Wk.T.reshape(2, 128, 256).transpose(1, 0, 2).astype(np.float16))


def _prep_wv(Wv):
    # [k, c] -> T [c, k] -> lhsT chunks [128, (ccl,kc), 128]
    return np.ascontiguousarray(
        Wv.T.reshape(2, 128, 2, 128).transpose(1, 0, 2, 3).reshape(128, 4, 128)
        .astype(np.float16))


def _prep_wout(Wo):
    # [o, C] with C=c*4+h -> Wo.T [C,o] -> head-major perm [h*256+c, o] -> chunks
    WoT = Wo.T.reshape(256, 4, 1024).transpose(1, 0, 2).reshape(1024, 1024)
    return np.ascontiguousarray(
        WoT.reshape(8, 128, 1024).transpose(1, 0, 2).astype(np.float16))


def make_in_maps(inputs):
    f = lambda x: np.asarray(x, dtype=np.float32)
    emb1, emb_all = f(inputs["emb1"]), f(inputs["emb_all"])
    embd1, emb_alld = f(inputs["embd1"]), f(inputs["emb_alld"])
    branch_w = [
        # (Wq-side, Wk-side, Wv-side, Wout-side)
        (_prep_wq(f(inputs["Wq"])), _prep_wk(f(inputs["Wkd0"])),
         _prep_wv(f(inputs["Wvd0"])), _prep_wout(f(inputs["Wout"]))),
        (_prep_wq(f(inputs["Wqd"])), _prep_wk(f(inputs["Wk0"])),
         _prep_wv(f(inputs["Wv0"])), _prep_wout(f(inputs["Woutd"]))),
    ]
    in_maps = []
    for core in range(8):
        b, br = core % 4, core // 4
        if br == 0:
            eq, ekv = emb1[b], emb_alld[b]
        else:
            eq, ekv = embd1[b], emb_all[b]
        wq, wk, wv, wo = branch_w[br]
        in_maps.append({
            "embq": _prep_embT(eq),
            "embkv": _prep_embT(ekv),
            "wq": wq, "wk": wk, "wv": wv, "wout": wo,
        })
    return in_maps


_NC_CACHE = {}


def get_nc(n_tokens=N):
    if n_tokens not in _NC_CACHE:
        _NC_CACHE[n_tokens] = build_nc(n_tokens)
    return _NC_CACHE[n_tokens]


def run_on_hw(in_maps, trace=False):
    _ensure_axon_hooks()
    from concourse.bass_utils import run_bass_kernel_spmd
    nc = get_nc()
    return run_bass_kernel_spmd(nc, in_maps, list(range(len(in_maps))), trace=trace)


def kernel(**inputs):
    res = run_on_hw(make_in_maps(inputs), trace=False)
    O1 = np.stack([res.results[b]["out"] for b in range(4)])
    Od1 = np.stack([res.results[4 + b]["out"] for b in range(4)])
    return O1, Od1



# revision 7
# speedup vs baseline: 1.0891x; 1.0891x over previous
"""Trainium2 Bass kernel for nn_Attention_org_cross_85074712199395.

Reference computes two fully independent cross-attention branches:
  branch 0: Q(emb1;Wq)   x Kd,Vd(emb_alld;Wkd0,Wvd0) -> O1  via Wout
  branch 1: Qd(embd1;Wqd) x K,V  (emb_all; Wk0, Wv0 ) -> Od1 via Woutd
Sharding: 8 cores = 4 batches x 2 branches. Zero collectives.

Algebraic refactor (scores contract over tokens, so weight rotations
commute out of the big GEMMs):
  T'[h] = ekv[h]^T eq[h]            [256b,256a]  contract n=4096 (the only
                                    big phase-A matmul; no Q/K projections)
  U[h]  = T'[h]^T Wk^T              [256a,256k]  tiny
  s[h]  = Wq[h] U[h]                [256c,256k]  tiny; = s_ref*sqrt(KV)
  p[h]  = softmax(inorm(s)) / den   (eps' = eps*KV absorbs the scale)
  M[h]  = p[h]^T W~[h]              [256k,1024o] tiny (W~[h][c,o]=Wout[o,4c+h])
  G[h]  = Wv^T M[h]                 [256j,1024o] tiny
  out   = sum_h ekv[h] G[h]         [4096,1024]  the only big phase-B matmul
This removes the Q/K/V projections, the ctx matmul, and all P transposes:
PE work drops from ~248us to ~155us. W~ and Wv are host-scaled by 64 and
the 2^-12 is removed in the output evacuation. All operands fp16 (fp32
PSUM); output is written fp16 and upcast on host.
"""

import sys
import types

import numpy as np

B, N, C, KV, H = 4, 4096, 1024, 1024, 4
Ch = C // H          # 256
EPS_ADJ = 1e-5 * KV  # InstanceNorm eps with the 1/sqrt(KV) score scale folded in
WSC = 64.0           # host scale on W~ and Wv_G; 2^-12 removed at output evac
NT = N // 128        # 32 token stripes


def _ensure_axon_hooks():
    """Inject antenv.axon_hooks (absent in this image) so trace=True works."""
    if "antenv.axon_hooks" in sys.modules:
        return
    try:
        import antenv  # noqa: F401
    except ImportError:
        return
    mod = types.ModuleType("antenv.axon_hooks")
    state = [None]
    mod.set_axon_ntff_profile_hook = lambda h: state.__setitem__(0, h)
    mod.get_axon_ntff_profile_hook = lambda: state[0]
    sys.modules["antenv.axon_hooks"] = mod
    try:
        from trn_agent_boot.trn_boot import _ntff_profile_via_ctypes

        mod.set_axon_ntff_profile_hook(
            _ntff_profile_via_ctypes("/opt/axon/libaxon_pjrt.so")
        )
    except Exception:
        pass


def build_nc(n_tokens=N):
    """Build + compile the per-core Bass program (SPMD-identical on all cores)."""
    import concourse.mybir as mybir
    import concourse.tile as tile
    from concourse import bacc

    f32 = mybir.dt.float32
    f16 = mybir.dt.float16
    Exp = mybir.ActivationFunctionType.Exp
    X = mybir.AxisListType.X
    nt = n_tokens // 128

    nc = bacc.Bacc("TRN2", target_bir_lowering=False, debug=False, num_devices=8)

    embq_d = nc.dram_tensor("embq", [n_tokens, 1024], f16, kind="ExternalInput").ap()
    embkvt_d = nc.dram_tensor("embkvt", [n_tokens, 1024], f16, kind="ExternalInput").ap()
    embkvc_d = nc.dram_tensor("embkvc", [128, 8, n_tokens], f16, kind="ExternalInput").ap()
    wq_d = nc.dram_tensor("wq", [128, 8, 256], f16, kind="ExternalInput").ap()
    wk_d = nc.dram_tensor("wk", [128, 2, 256], f16, kind="ExternalInput").ap()
    wv_d = nc.dram_tensor("wv", [128, 2, 256], f16, kind="ExternalInput").ap()
    wt_d = nc.dram_tensor("wt", [128, 8, 1024], f16, kind="ExternalInput").ap()
    out_d = nc.dram_tensor("out", [n_tokens, 1024], f16, kind="ExternalOutput").ap()

    with tile.TileContext(nc) as tc:
        with tc.tile_pool(name="weights", bufs=1) as wpool, \
             tc.tile_pool(name="gsb", bufs=1) as gpool, \
             tc.tile_pool(name="tu_sb", bufs=10) as tu_sb, \
             tc.tile_pool(name="stat", bufs=1) as stat_pool:

            ones_col = wpool.tile([128, 1], f32)
            nc.vector.memset(ones_col[:], 1.0)
            ones_row = wpool.tile([1, 128], f32)
            nc.vector.memset(ones_row[:], 1.0)
            warm_sb = wpool.tile([128, 128], f32)
            nc.vector.memset(warm_sb[:], 0.0)

            wq = wpool.tile([128, 8, 256], f16)
            wk = wpool.tile([128, 2, 256], f16)
            wv = wpool.tile([128, 2, 256], f16)
            wt = wpool.tile([128, 8, 1024], f16)
            gsb = gpool.tile([128, 8, 1024], f16)
            dens = [stat_pool.tile([128, 4], f32, name=f"den{i}", tag=f"den{i}")
                    for i in range(4)]

            # ============ phase A: stream tokens, accumulate T' =============
            tsbs = []
            with tc.tile_pool(name="tp_ps", bufs=1, space="PSUM") as tp_ps:
                tps = [tp_ps.tile([128, 2, 256], f32, name=f"tp{h}", tag=f"tp{h}")
                       for h in range(H)]
                with tc.tile_pool(name="warm_ps", bufs=1, space="PSUM") as warm_pool, \
                     tc.tile_pool(name="eq_st", bufs=5) as eq_pool, \
                     tc.tile_pool(name="ekv_st", bufs=5) as ekv_pool:
                    # keep PE busy while the first token blocks stream in
                    wps = warm_pool.tile([128, 512], f32)
                    for w in range(10):
                        nc.tensor.matmul(wps[:, 0:128], warm_sb[:], warm_sb[:],
                                         start=(w == 0), stop=(w == 9))
                    for blk in range(nt):
                        sl = slice(blk * 128, (blk + 1) * 128)
                        eq = eq_pool.tile([128, 1024], f16)
                        ekv = ekv_pool.tile([128, 1024], f16)
                        nc.sync.dma_start(eq[:], embq_d[sl, :])
                        nc.sync.dma_start(ekv[:], embkvt_d[sl, :])
                        if blk == 3:
                            # weights are first needed in the mid phase; keep
                            # them out of the startup DMA window
                            nc.sync.dma_start(wq[:], wq_d[:])
                            nc.sync.dma_start(wk[:], wk_d[:])
                            nc.sync.dma_start(wv[:], wv_d[:])
                        if blk == 6:
                            nc.sync.dma_start(wt[:], wt_d[:])
                        for h in range(H):
                            for bc in range(2):
                                nc.tensor.matmul(
                                    tps[h][:, bc, :],
                                    ekv[:, h * 256 + bc * 128:h * 256 + (bc + 1) * 128],
                                    eq[:, h * 256:(h + 1) * 256],
                                    start=(blk == 0 and bc == 0),
                                    stop=(blk == nt - 1 and bc == 1))

                # T' -> SBUF fp16 (inside tp_ps scope; frees 4 banks after)
                for h in range(H):
                    tsb = tu_sb.tile([128, 2, 256], f16, name="tsb", tag="tsb")
                    nc.vector.tensor_copy(tsb[:], tps[h][:])
                    tsbs.append(tsb)

            # ============ mid phase: rotations, softmax, M, G ===============
            usbs = []
            with tc.tile_pool(name="u_ps", bufs=2, space="PSUM") as u_ps:
                # U[h] = T'[h]^T Wk^T
                for h in range(H):
                    up = u_ps.tile([128, 2, 256], f32)
                    for ac in range(2):
                        for i, bc in enumerate((0, 1)):
                            nc.tensor.matmul(
                                up[:, ac, :],
                                tsbs[h][:, bc, ac * 128:(ac + 1) * 128],
                                wk[:, bc, :],
                                start=(ac == 0 and i == 0),
                                stop=(ac == 1 and i == 1))
                    usb = tu_sb.tile([128, 2, 256], f16, name="usb", tag="usb")
                    nc.vector.tensor_copy(usb[:], up[:])
                    usbs.append(usb)

            with tc.tile_pool(name="s_ps", bufs=1, space="PSUM") as s_ps, \
                 tc.tile_pool(name="m_sb", bufs=4) as m_sb, \
                 tc.tile_pool(name="p_sb", bufs=4) as p_pool, \
                 tc.tile_pool(name="stat4", bufs=4) as stat4_pool, \
                 tc.tile_pool(name="scr", bufs=1) as scr_pool:
                # s[h] = Wq[h] U[h]
                s_tiles = []
                for h in range(H):
                    sp = s_ps.tile([128, 2, 256], f32, name=f"s{h}", tag=f"s{h}")
                    for cc in range(2):
                        for i, ac in enumerate((0, 1)):
                            nc.tensor.matmul(
                                sp[:, cc, :],
                                wq[:, 2 * h + ac, cc * 128:(cc + 1) * 128],
                                usbs[h][:, ac, :],
                                start=(cc == 0 and i == 0),
                                stop=(cc == 1 and i == 1))
                    s_tiles.append(sp)

                # --- stacked inorm stats for all 4 heads ---
                scratch = scr_pool.tile([128, 512], f32)
                stat_all = stat4_pool.tile([128, 16], f32)
                for h in range(H):
                    for cc in range(2):
                        s_ap = s_tiles[h][:, cc, :]
                        nc.vector.reduce_sum(stat_all[:, h * 4 + cc:h * 4 + cc + 1],
                                             s_ap, axis=X)
                        nc.scalar.activation(
                            scratch[:, cc * 256:(cc + 1) * 256], s_ap,
                            mybir.ActivationFunctionType.Square,
                            accum_out=stat_all[:, h * 4 + 2 + cc:h * 4 + 3 + cc])
                tot_ctx = tc.tile_pool(name="tot_ps", bufs=1, space="PSUM")
                tot_ps_pool = tot_ctx.__enter__()
                tb = tot_ps_pool.tile([128, 16], f32, name="tb", tag="tb")
                nc.tensor.matmul(tb[:1, 0:16], ones_col[:], stat_all[:],
                                 start=True, stop=True)
                sc = stat4_pool.tile([1, 40], f32)
                nc.vector.tensor_copy(sc[:1, 0:16], tb[:1, 0:16])
                inv = 1.0 / (256.0 * 256.0)
                # cols h*4+{0,1,2,3} = sum0,sum1,sq0,sq1
                # -> [msum,qsum]x4 at 16:24, [mean,Ex2]x4 at 24:32
                nc.vector.tensor_add(sc[:1, 16:24], sc[:1, 0:16:2], sc[:1, 1:16:2])
                nc.vector.tensor_scalar_mul(sc[:1, 24:32], sc[:1, 16:24], inv)
                nc.vector.tensor_mul(sc[:1, 32:36], sc[:1, 24:32:2], sc[:1, 24:32:2])
                nc.vector.tensor_sub(sc[:1, 36:40], sc[:1, 25:32:2], sc[:1, 32:36])
                sc2 = stat4_pool.tile([1, 16], f32)
                nc.vector.tensor_scalar_add(sc2[:1, 0:4], sc[:1, 36:40], EPS_ADJ)
                nc.scalar.sqrt(sc2[:1, 4:8], sc2[:1, 0:4])
                nc.vector.reciprocal(sc2[:1, 8:12], sc2[:1, 4:8])          # rs x4
                nc.vector.tensor_mul(sc2[:1, 12:16], sc[:1, 24:32:2], sc2[:1, 8:12])
                nc.vector.tensor_scalar_mul(sc2[:1, 12:16], sc2[:1, 12:16], -1.0)
                # broadcast [rs x4, -rs*m x4] to all partitions via PE
                nc.tensor.matmul(tb[:, 0:8], ones_row[:], sc2[:1, 8:16],
                                 start=True, stop=True)
                bc_t = stat4_pool.tile([128, 8], f32)
                nc.vector.tensor_copy(bc_t[:], tb[:, 0:8])
                tot_ctx.__exit__(None, None, None)  # free tb's PSUM bank

                m_ps_ctx = tc.tile_pool(name="m_ps", bufs=2, space="PSUM")
                g_ps_ctx = tc.tile_pool(name="g_ps", bufs=2, space="PSUM")
                m_ps = m_ps_ctx.__enter__()
                g_ps = g_ps_ctx.__enter__()
                # per head: exp -> p (scaled by 1/den) -> M -> G
                for h in range(H):
                    e_t = p_pool.tile([128, 2, 256], f16, name="e", tag="e")
                    p_t = p_pool.tile([128, 2, 256], f16, name="p", tag="p")
                    den = dens[h]
                    for cc in range(2):
                        nc.scalar.activation(
                            e_t[:, cc, :], s_tiles[h][:, cc, :],
                            Exp, bias=bc_t[:, 4 + h:5 + h], scale=bc_t[:, h:h + 1],
                            accum_out=den[:, cc:cc + 1])
                        nc.vector.reciprocal(den[:, 2 + cc:3 + cc], den[:, cc:cc + 1])
                        nc.vector.tensor_scalar_mul(p_t[:, cc, :], e_t[:, cc, :],
                                                    den[:, 2 + cc:3 + cc])
                    # M[h] = p^T W~[h]  -> msb [128k, 2kc, 1024o]
                    msb = m_sb.tile([128, 2, 1024], f16, name="msb", tag="msb")
                    for kc in range(2):
                        for oh in range(2):
                            mp = m_ps.tile([128, 512], f32, name="mp", tag="mp")
                            for i, cc in enumerate((0, 1)):
                                nc.tensor.matmul(
                                    mp[:],
                                    p_t[:, cc, kc * 128:(kc + 1) * 128],
                                    wt[:, 2 * h + cc, oh * 512:(oh + 1) * 512],
                                    start=(i == 0), stop=(i == 1))
                            nc.vector.tensor_copy(
                                msb[:, kc, oh * 512:(oh + 1) * 512], mp[:])
                    # G[h] = Wv^T M[h] -> gsb[:, 2h+jc, :]
                    for jc in range(2):
                        for oh in range(2):
                            gp = g_ps.tile([128, 512], f32, name="gp", tag="gp")
                            for i, kc in enumerate((0, 1)):
                                nc.tensor.matmul(
                                    gp[:],
                                    wv[:, kc, jc * 128:(jc + 1) * 128],
                                    msb[:, kc, oh * 512:(oh + 1) * 512],
                                    start=(i == 0), stop=(i == 1))
                            nc.vector.tensor_copy(
                                gsb[:, 2 * h + jc, oh * 512:(oh + 1) * 512], gp[:])
                g_ps_ctx.__exit__(None, None, None)
                m_ps_ctx.__exit__(None, None, None)

            # ============ phase B: out = sum_h ekv[h] G[h] ==================
            with tc.tile_pool(name="ekvc_st", bufs=6) as ekvc_pool, \
                 tc.tile_pool(name="o_ps", bufs=4, space="PSUM") as o_ps, \
                 tc.tile_pool(name="o_sb", bufs=4) as o_sb:
                ekvc_tiles = {}
                PREF = 4
                for t in range(min(PREF, nt)):
                    ek = ekvc_pool.tile([128, 8, 128], f16, name="ekvc", tag="ekvc")
                    nc.sync.dma_start(ek[:], embkvc_d[:, :, t * 128:(t + 1) * 128])
                    ekvc_tiles[t] = ek
                for t in range(nt):
                    ek = ekvc_tiles.pop(t)
                    if t + PREF < nt:
                        ek2 = ekvc_pool.tile([128, 8, 128], f16, name="ekvc", tag="ekvc")
                        nc.sync.dma_start(
                            ek2[:], embkvc_d[:, :, (t + PREF) * 128:(t + PREF + 1) * 128])
                        ekvc_tiles[t + PREF] = ek2
                    ot = o_sb.tile([128, 1024], f16)
                    for oh in range(2):
                        op = o_ps.tile([128, 512], f32)
                        for j in range(8):
                            nc.tensor.matmul(
                                op[:], ek[:, j, :],
                                gsb[:, j, oh * 512:(oh + 1) * 512],
                                start=(j == 0), stop=(j == 7))
                        nc.vector.tensor_scalar_mul(ot[:, oh * 512:(oh + 1) * 512],
                                                    op[:], 1.0 / (WSC * WSC))
                    nc.sync.dma_start(out_d[t * 128:(t + 1) * 128, :], ot[:])

    nc.compile()
    return nc


# ---------------- host-side data prep ----------------

def _prep_embT(e):
    # [nt, 1024] -> [128, 8, nt]: partition p, chunk cc -> channel cc*128+p
    return np.ascontiguousarray(
        e.T.reshape(8, 128, -1).transpose(1, 0, 2).astype(np.float16))


def _prep_wq(Wq):
    # lhsT layout [128a, (h,ac), 256c]: wq[p, 2h+ac, c] = Wq[h][c, ac*128+p]
    WqT = Wq.transpose(0, 2, 1)  # [h, a, c]
    return np.ascontiguousarray(
        WqT.reshape(4, 2, 128, 256).transpose(2, 0, 1, 3)
        .reshape(128, 8, 256).astype(np.float16))


def _prep_wk(Wk):
    # rhs layout [128b, bc, 256k]: wk[p, bc, k] = Wk[k, bc*128+p]
    return np.ascontiguousarray(
        Wk.T.reshape(2, 128, 256).transpose(1, 0, 2).astype(np.float16))


def _prep_wv(Wv):
    # lhsT layout [128k, kc, 256j]: wv[p, kc, j] = Wv[kc*128+p, j] * WSC
    return np.ascontiguousarray(
        (Wv * WSC).reshape(2, 128, 256).transpose(1, 0, 2).astype(np.float16))


def _prep_wt(Wo):
    # W~[h][c,o] = Wout[o, c*4+h];  wt[p, 2h+cc, o] = W~[h][cc*128+p, o] * WSC
    WoT = Wo.T  # [C, o], C = c*4+h
    Wth = WoT.reshape(256, 4, 1024).transpose(1, 0, 2)  # [h, c, o]
    return np.ascontiguousarray(
        (Wth.reshape(4, 2, 128, 1024).transpose(2, 0, 1, 3)
         .reshape(128, 8, 1024) * WSC).astype(np.float16))


def make_in_maps(inputs):
    f = lambda x: np.asarray(x, dtype=np.float32)
    emb1, emb_all = f(inputs["emb1"]), f(inputs["emb_all"])
    embd1, emb_alld = f(inputs["embd1"]), f(inputs["emb_alld"])
    branch_w = [
        # (Wq-side, Wk-side, Wv-side, Wout-side)
        (_prep_wq(f(inputs["Wq"])), _prep_wk(f(inputs["Wkd0"])),
         _prep_wv(f(inputs["Wvd0"])), _prep_wt(f(inputs["Wout"]))),
        (_prep_wq(f(inputs["Wqd"])), _prep_wk(f(inputs["Wk0"])),
         _prep_wv(f(inputs["Wv0"])), _prep_wt(f(inputs["Woutd"]))),
    ]
    in_maps = []
    for core in range(8):
        b, br = core % 4, core // 4
        if br == 0:
            eq, ekv = emb1[b], emb_alld[b]
        else:
            eq, ekv = embd1[b], emb_all[b]
        wq, wk, wv, wt = branch_w[br]
        in_maps.append({
            "embq": np.ascontiguousarray(eq.astype(np.float16)),
            "embkvt": np.ascontiguousarray(ekv.astype(np.float16)),
            "embkvc": _prep_embT(ekv),
            "wq": wq, "wk": wk, "wv": wv, "wt": wt,
        })
    return in_maps


_NC_CACHE = {}


def get_nc(n_tokens=N):
    if n_tokens not in _NC_CACHE:
        _NC_CACHE[n_tokens] = build_nc(n_tokens)
    return _NC_CACHE[n_tokens]


def run_on_hw(in_maps, trace=False):
    _ensure_axon_hooks()
    from concourse.bass_utils import run_bass_kernel_spmd
    nc = get_nc()
    return run_bass_kernel_spmd(nc, in_maps, list(range(len(in_maps))), trace=trace)


def kernel(**inputs):
    res = run_on_hw(make_in_maps(inputs), trace=False)
    O1 = np.stack([res.results[b]["out"].astype(np.float32) for b in range(4)])
    Od1 = np.stack([res.results[4 + b]["out"].astype(np.float32) for b in range(4)])
    return O1, Od1
